# revision 18
# baseline (speedup 1.0000x reference)
"""DisparityConv kernel for 8 Trainium2 NeuronCores.

Full inputs: x[8,32,256,512] f32, W[64,32,3,3] f32, bias[64] f32.
Data-parallel over batch: core i computes x[i] -> out[i] [64,256,512].

Per-core pipeline:
  xe (bf16, channel-PAIR interleaved along w, width-extended by S for the
  circular roll) prepared host-side: free index 2w+e holds channel 2cp+e.
  Partition layout (j,cp): 8 stagger-replicas (shift j+1 baked, in elems
  2(j+1)) x 16 channel-pairs. Per shift-group g (delta=8g): the custom DVE
  op PS_ABSDIFF computes |XR[2w] - X4[2w]| + |XR[2w+1] - X4[2w+1]| -- a
  channel-PAIR abs-diff sum -- at the same 2 elem/lane/cycle input rate as
  a plain 2x abs-diff, emitting HALF the bytes. Shifts s=8g+j+1 live on
  partition group j.
  Channel mean + shift departition: mask matmul on PE (half the moving
  columns of the unpaired form) -> psum[(jrep,s), w], jrep replicas 0..2
  become the kh=0..2 blocks of the conv's K=96 operand.
  ScalarE casts psum[0:96] -> Dstage bf16; 3 SBUF-SBUF DMAs scatter the
  three kh blocks onto a DIAGONAL ring D3h (block kh of diff row r lands at
  slot (r+1-kh)%R), so conv for output row rr reads the single uniform slot
  rr%R and gets rows rr-1, rr, rr+1 stacked on partitions: 3 dense K=96
  matmuls (one per kw, rhs w-offset) accumulating in PSUM, col-split by row
  parity. Due convs run at the head of each row-batch (their ring slots
  were scattered CLAG rows ago) so they never serialize behind the current
  batch's evict+scatter chain. Bias added by ScalarE on PSUM eviction; f32
  DMA out.
"""
import sys

sys.path.insert(0, "/opt/trn_rl_repo")

import numpy as np
import ml_dtypes

import concourse.bass as bass  # noqa: F401
import concourse.tile as tile
from concourse import bacc, mybir
from concourse import bass_utils
from concourse import dve_ops
from concourse.dve_ops import DveOp
from concourse.dve_spec import Spec, Src0, Src1, Bin, lower, _has_src1
from concourse.dve_uop import (
    DveOpSpec, UopConfig, AluOp as UAluOp, AluInp, DelayInp, InpSel,
    OutSel, OutPath, Trigger, ENABLE,
)

F32 = mybir.dt.float32
BF16 = mybir.dt.bfloat16
Alu = mybir.AluOpType
Act = mybir.ActivationFunctionType

B = 8
C = 32
S = 32
O = 64
G4 = 4          # shift groups (8 shifts per group)
FULL_H, FULL_W = 256, 512
N_CORES = 8


def _uops_2x_pairsum():
    """2X_1PORT uop program for |a0-b0| + |a1-b1| over packed bf16 pairs.

    Mirrors the stock tensor_tensor 2x program's input routing (one 32-bit
    word per port per cycle = two packed bf16; HI halves ride delay chains
    1/2), computes both ABSOLUTE_DIFFs, then block2 ADDs them and the write
    stage emits the SUM in BOTH 16-bit halves (duplicated -- keeping the
    out AP the same size as the inputs, which the engine's sequencing
    requires; consumers read it at stride 2)."""
    u = UopConfig()
    for lane, src in enumerate((InpSel.SRC_0, InpSel.SRC_1,
                                InpSel.SRC_0_HI, InpSel.SRC_1_HI)):
        u.inp[lane] = src
        u.inp_enable[lane] = ENABLE
    u.require_inp0 = ENABLE
    u.require_inp1 = ENABLE
    u.trigger = (Trigger.SRC_TENSOR_DONE, Trigger.NONE, Trigger.NONE)
    u.out[OutPath.WR0_LO] = OutSel.ALU_OUT
    u.out_enable[OutPath.WR0_LO] = ENABLE
    u.out[OutPath.WR0_HI] = OutSel.DELAY_0
    u.out_enable[OutPath.WR0_HI] = ENABLE
    dp = u.datapath_config
    dp[0].enable_alu(UAluOp.ABSOLUTE_DIFF, AluInp.PREV_ALU_OUT,
                     AluInp.PREV_DELAY_0)
    dp[0].pass_through_delay(1, 2)
    dp[1].enable_alu(UAluOp.ABSOLUTE_DIFF, AluInp.PREV_DELAY_1,
                     AluInp.PREV_DELAY_2)
    dp[1].enable_delay_from_src(DelayInp.PREV_ALU_OUT, 0)
    dp[2].enable_alu(UAluOp.ADD, AluInp.PREV_ALU_OUT, AluInp.PREV_DELAY_0)
    for b in range(3, 8):
        dp[b].pass_through_alu()
        dp[b].pass_through_delay(0)
    return [u]


def _register_pairsum():
    if "PS_ABSDIFF_ANT" in dve_ops._SUB_OPCODE_FOR_NAME:
        return dve_ops._PS_ABSDIFF_ANT_OP
    # Interp/CoreSim semantics: pairwise |a-b| sum along the last free dim.
    # The generated 1x program (from the elementwise body) is WRONG for this
    # op -- all APs used are 16-bit/step-1/4B-aligned so the engine always
    # picks the (hand-written) 2x program; a 1x fallback would fail the
    # harness rel-err gate loudly rather than silently.
    def _ref(in0, in1, s0, s1, imm2):
        t = np.abs(in0.astype(np.float32)
                   - in1.astype(np.float32).reshape(in0.shape))
        s = t.reshape(*in0.shape[:-1], in0.shape[-1] // 2, 2).sum(-1)
        out = np.repeat(s, 2, axis=-1)
        return out

    spec = Spec(
        body=Bin(UAluOp.ABSOLUTE_DIFF, Src0, Src1),
        reference=_ref,
    )
    row = dve_ops._CUSTOM_DVE_ROW_BASE + len(dve_ops.OPS)
    assert row < 0x20
    op = DveOp("PS_ABSDIFF_ANT", spec, subdim=False, uops_sha={})
    dve_ops._SUB_OPCODE_FOR_NAME["PS_ABSDIFF_ANT"] = row
    dve_ops.OPS.append(op)
    dve_ops.CUSTOM_DVE_SPECS["PS_ABSDIFF_ANT"] = spec
    compiled = DveOpSpec(
        name="PS_ABSDIFF_ANT", opcode=row, uops=lower(spec, ver="v3"),
        uops_2x=_uops_2x_pairsum(), perf_max=1, rd1_en=_has_src1(spec),
    )
    op.uops_sha["v3"] = compiled.sha("v3")
    dve_ops._COMPILE_CACHE[("PS_ABSDIFF_ANT", "v3")] = compiled
    dve_ops._PS_ABSDIFF_ANT_OP = op
    # byte-36[7:6] perf_max must be nonzero on the *instruction* for the
    # engine to consider perf modes; _custom_dve hardcodes 0.
    if not getattr(bass.bass_isa, "_ant_pairsum_perfmax_patch", False):
        orig_ctor = bass.bass_isa.InstCustomDveAnt

        def _ctor(**kw):
            if kw.get("op_name") == "PS_ABSDIFF_ANT":
                kw["perf_max"] = 1
            return orig_ctor(**kw)

        bass.bass_isa.InstCustomDveAnt = _ctor
        bass.bass_isa._ant_pairsum_perfmax_patch = True
    return op


def _build_nc(H=FULL_H, W=FULL_W, hb=8, R=20, CLAG=13, num_devices=N_CORES):
    psum_op = _register_pairsum()
    WE = W + S
    W2 = 2 * W        # pair-interleaved row elems consumed per shift window
    WE2 = 2 * WE      # pair-interleaved staged row elems
    nc = bacc.Bacc("TRN2", target_bir_lowering=False, debug=False,
                   num_devices=num_devices)

    # xe: [16 cpairs, H+1 (one junk pad row), 2*WE] pair-interleaved bf16
    xe = nc.dram_tensor("xe", [16, H + 1, WE2], BF16, kind="ExternalInput").ap()
    xe_flat = xe.rearrange("c h w -> c (h w)")
    masks = nc.dram_tensor("masks", [G4, 128, 128], BF16, kind="ExternalInput").ap()
    convw = nc.dram_tensor("convw", [3, 96, O], BF16, kind="ExternalInput").ap()
    bias2 = nc.dram_tensor("bias2", [128, 1], F32, kind="ExternalInput").ap()
    # output stored as [h%2, o, h//2, w] so a row-pair batch [128=(h2,o), k, w]
    # is one contiguous-partition DMA; host transposes back
    out2 = nc.dram_tensor("out2", [2, O, H // 2, W], F32,
                          kind="ExternalOutput").ap()
    out2v = out2.rearrange("a o h w -> (a o) h w")

    assert H % hb == 0 and H % 2 == 0
    nblk = H // hb

    with tile.TileContext(nc) as tc:
        with (
            tc.tile_pool(name="const", bufs=1) as constp,
            tc.tile_pool(name="stage", bufs=2) as stagep,
            tc.tile_pool(name="ap", bufs=5) as apool,
            tc.tile_pool(name="dst", bufs=3) as dstp,
            tc.tile_pool(name="d4p", bufs=1) as d4p,
            tc.tile_pool(name="outp", bufs=2) as outp,
            tc.tile_pool(name="psd", bufs=2, space="PSUM") as psdp,
            tc.tile_pool(name="pso", bufs=4, space="PSUM") as psop,
        ):
            maskT = constp.tile([128, G4 * 128], BF16)
            for g in range(G4):
                nc.sync.dma_start(maskT[:, g * 128:(g + 1) * 128], masks[g])
            WT = constp.tile([96, 3 * O], BF16)
            for kw in range(3):
                nc.sync.dma_start(WT[:, kw * O:(kw + 1) * O], convw[kw])
            biasT = constp.tile([128, 1], F32)
            nc.sync.dma_start(biasT[:], bias2[:])

            d3 = d4p.tile([96, R * (W + 2)], BF16)
            d3v = d3.rearrange("p (r w) -> p r w", w=W + 2)
            for sl in range(R):
                nc.vector.memset(d3v[:, sl, :], 0)

            def conv_pairs(p0s):
                # interleave the matmul streams of up to 2 row-pairs so one
                # accumulation chain's drain hides under the other's fill
                psos = {p0: psop.tile([128, W], F32, tag="pso", name=f"pso{p0}")
                        for p0 in p0s}
                for half in (0, 1):
                    for kw in (0, 1, 2):
                        for p0 in p0s:
                            rr = p0 + half
                            nc.tensor.matmul(
                                psos[p0][64 * half:64 * half + 64, :],
                                WT[:, kw * O:kw * O + O],
                                d3v[:, rr % R, kw:kw + W],
                                start=(kw == 0), stop=(kw == 2),
                                tile_position=(0, 64 * half),
                            )
                # both pairs of a call share one staging tile so the store
                # is a single wide DMA; stores ride the scalar HWDGE ring
                ot = outp.tile([128, 2 * W], F32)
                otv = ot.rearrange("p (k w) -> p k w", w=W)
                for ki, p0 in enumerate(p0s):
                    # bias-add eviction on ACT (Identity spline + per-partition
                    # bias AP)
                    nc.scalar.activation(otv[:, ki, :], psos[p0][:],
                                         Act.Identity, bias=biasT[:])
                k0 = p0s[0] // 2
                nc.scalar.dma_start(out2v[:, k0:k0 + len(p0s), :],
                                    otv[:, 0:len(p0s), :])

            def load_block(blk):
                h0 = blk * hb
                # full-row (stride WE2) staging tiles loaded as flat
                # contiguous slices of xe; the j+1 stagger (2(j+1) elems in
                # the pair-interleaved layout) is baked into the flat source
                # offset. x4 rides the sync ring; xr is split across the
                # gpsimd and scalar rings (4.4 MB/block of staging traffic
                # needs three rings to keep the DVE fed).
                x4 = stagep.tile([128, hb * WE2], BF16, tag="x4",
                                 name=f"x4_{blk}")
                xr = stagep.tile([128, hb * WE2], BF16, tag="xr",
                                 name=f"xr_{blk}")
                n = hb * WE2
                for j in range(8):
                    nc.sync.dma_start(
                        x4[16 * j:16 * j + 16, :],
                        xe_flat[:, WE2 * h0:WE2 * h0 + n])
                    eng = nc.gpsimd if j % 2 == 0 else nc.scalar
                    eng.dma_start(
                        xr[16 * j:16 * j + 16, :],
                        xe_flat[:, WE2 * h0 + 2 * (j + 1):
                                WE2 * h0 + 2 * (j + 1) + n])
                return x4, xr

            def produce_block(blk, staged):
                x4, xr = staged
                x4v = x4.rearrange("p (h w) -> p h w", w=WE2)
                xrv = xr.rearrange("p (h w) -> p h w", w=WE2)
                x4w = x4v[:, :, 0:W2]
                avs = []
                for g in range(G4):
                    # output duplicated per pair: [.., 2w] == [.., 2w+1] ==
                    # pair-sum; mask matmuls read it at stride 2
                    a = apool.tile([128, hb * W2], BF16, tag="a",
                                   name=f"a_{blk}_{g}")
                    av = a.rearrange("p (h w) -> p h w", w=W2)
                    in0 = xrv[:, :, 16 * g:16 * g + W2]
                    nc.vector._custom_dve(psum_op, out=av, in0=in0, in1=x4w)
                    avs.append(av)
                return avs

            staged = {b: load_block(b) for b in range(min(3, nblk))}
            next_avs = produce_block(0, staged.pop(0))
            for blk in range(nblk):
                h0 = blk * hb
                avs = next_avs
                if blk + 3 < nblk:
                    staged[blk + 3] = load_block(blk + 3)
                if blk + 1 < nblk:
                    next_avs = produce_block(blk + 1, staged.pop(blk + 1))
                for rl in range(0, hb, 4):
                    r = h0 + rl + 3
                    # due convs run FIRST: their ring slots were scattered
                    # blocks ago, and emitting them before this batch's
                    # scatter keeps them off its (coarse) d3 dependency
                    due = [p0 for p0 in (r - CLAG, r - CLAG + 2)
                           if 0 <= p0 <= H - 4]
                    if due:
                        conv_pairs(due)
                    # four rows' mask-matmul chains interleaved (independent
                    # accumulators) so drains overlap fills; paired rows
                    # share a [128, 2W] tile so eviction is 2 wide ACT
                    # copies instead of 4 narrow ones
                    psd2 = [psdp.tile([128, 2 * W], F32, tag="psd",
                                      name=f"psd{blk}_{rl}_{h2}")
                            for h2 in range(2)]
                    for g in range(G4):
                        for q in range(4):
                            nc.tensor.matmul(
                                psd2[q // 2][:, (q % 2) * W:(q % 2) * W + W],
                                maskT[:, g * 128:(g + 1) * 128],
                                avs[g][:, rl + q, 0:W2:2],
                                start=(g == 0), stop=(g == G4 - 1),
                            )
                    ds4 = dstp.tile([96, 4 * W], BF16, tag="ds4",
                                    name=f"ds4_{blk}_{rl}")
                    ds4v = ds4.rearrange("p (a w) -> p a w", w=W)
                    for h2 in range(2):
                        nc.scalar.copy(ds4[:, h2 * 2 * W:(h2 + 1) * 2 * W],
                                       psd2[h2][0:96, :])
                    # batched diagonal-ring scatter: rows r-3..r, per kh
                    # block kh of diff row q lands at slot (q+1-kh)%R
                    r0 = r - 3
                    for kh in range(3):
                        s0 = (r0 + 1 - kh) % R
                        n1 = min(4, R - s0)
                        for (a0, sl0, cnt) in (((0, s0, n1),) if n1 == 4 else
                                               ((0, s0, n1), (n1, 0, 4 - n1))):
                            nc.gpsimd.dma_start(
                                d3v[32 * kh:32 * kh + 32,
                                    sl0:sl0 + cnt, 1:W + 1],
                                ds4v[32 * kh:32 * kh + 32,
                                     a0:a0 + cnt, :])
            # zero the kh=2 slot that would hold (nonexistent) diff row H
            nc.vector.memset(d3v[64:96, (H - 1) % R, 1:W + 1], 0)
            # flush conv rows still pending behind the CLAG lag
            p0 = (H - 1) - CLAG + 4
            while p0 <= H - 4:
                conv_pairs([p0, p0 + 2] if p0 + 2 <= H - 4 else [p0])
                p0 += 4
            if p0 == H - 2:
                conv_pairs([H - 2])

    nc.compile()
    return nc


_NC_CACHE = {}


def _get_nc():
    if "nc" not in _NC_CACHE:
        _NC_CACHE["nc"] = _build_nc()
    return _NC_CACHE["nc"]


def host_prep_shared(Wc, bias):
    bf16 = ml_dtypes.bfloat16
    masks = np.zeros((G4, 128, 128), np.float32)
    for g in range(G4):
        for j in range(8):
            for jr in range(4):
                masks[g, 16 * j:16 * j + 16, 32 * jr + 8 * g + j] = 1.0 / C
    masks = masks.astype(bf16)
    # convw[kw, 32*kh + s, o] = Wc[o, s, kh, kw]
    convw = np.ascontiguousarray(
        Wc.transpose(3, 2, 1, 0).reshape(3, 96, O)).astype(bf16)
    bias2 = np.concatenate([bias, bias]).reshape(128, 1).astype(np.float32)
    return masks, convw, bias2


def kernel(x, W, bias, _trace=False, _tmpdir=None):
    """x:[8,32,256,512] f32, W:[64,32,3,3] f32, bias:[64] f32 -> [8,64,256,512]."""
    nc = _get_nc()
    bf16 = ml_dtypes.bfloat16
    masks, convw, bias2 = host_prep_shared(np.asarray(W, np.float32),
                                           np.asarray(bias, np.float32))
    x = np.asarray(x, np.float32)
    # width-extend for the circular roll, then interleave channel pairs
    # along w: xe[cp, h, 2w+e] = x[2cp+e, h, w]
    xw = np.concatenate([x, x[:, :, :, :S]], axis=3)          # [B, C, H, WE]
    Bn, _, Hn, WEn = xw.shape
    xp = xw.reshape(Bn, 16, 2, Hn, WEn).transpose(0, 1, 3, 4, 2)
    xp = np.ascontiguousarray(xp).reshape(Bn, 16, Hn, 2 * WEn).astype(bf16)
    # one junk pad row per cpair (flat shifted loads overrun the last block)
    xe_all = np.concatenate([xp, np.zeros_like(xp[:, :, :1, :])], axis=2)
    in_maps = [
        {"xe": xe_all[i], "masks": masks, "convw": convw, "bias2": bias2}
        for i in range(N_CORES)
    ]
    kw = {}
    if _trace:
        kw = dict(trace=True, tmpdir=_tmpdir)
    res = bass_utils.run_bass_kernel_spmd(
        nc, in_maps, core_ids=list(range(N_CORES)), **kw)
    out = np.stack(
        [np.ascontiguousarray(
            res.results[i]["out2"].transpose(1, 2, 0, 3).reshape(O, FULL_H,
                                                                 FULL_W))
         for i in range(N_CORES)], axis=0)
    if _trace:
        kernel.last_exec_time_ns = res.exec_time_ns
        kernel.last_results = res
    return out
